# revision 14
# baseline (speedup 1.0000x reference)
"""Trainium2 Bass kernel for single-token decode attention (NaiveAttention).

Math (per reference):
  q = x @ W_Q.T ; k_new = x @ W_K.T ; v_new = x @ W_V.T        (each (32, 128))
  k_cache[seq, pos] = k_new ; v_cache[seq, pos] = v_new
  K = k_cache[seq, :pos+1] ; V = v_cache[seq, :pos+1]
  scores = (q . K) / sqrt(128) ; attn = softmax(scores)
  out = (attn . V) @ W_O.T                                     ((1, 1, 4096))

Sharding: tensor-parallel over heads. 8 cores x 4 heads. W_Q/W_K/W_V are
sharded column-wise (after transpose), W_O row-wise; each core computes a
partial (4096,) output vector and the host sums the 8 partials.

Device strategy: every large matrix (weight shards, K, V, W_O) streams
through the tensor engine as the *moving* operand in float32r mode
(1 cycle/column at N=512, vs 4 cycles/column for fp32 and ~700ns/tile for
fp32 LDWEIGHTS+matmul pairs), with single-column stationary vectors.
float32r consumes raw fp32 bytes (measured ~2e-4 matmul rel-err vs 2.7e-3
for bf16). Attention probabilities are transposed to columns with small PE
transposes, then A@V runs head-batched: lhsT = p(128s x 4heads),
rhs = [V_h0|V_h1|V_h2|V_h3](128s x 512) -> the diagonal 128-blocks of the
(4,512) result are the per-head outputs (extra PE columns are free).
"""

import sys

if "/opt/trn_rl_repo" not in sys.path:
    sys.path.insert(0, "/opt/trn_rl_repo")

import numpy as np

D_MODEL = 4096
N_HEADS = 32
D_K = 128
S = 4096          # pos + 1 for the compiled fast path
N_CORES = 8
HPC = N_HEADS // N_CORES          # heads per core = 4
MPC = HPC * D_K                   # model dims per core = 512
INV_SQRT_DK = 1.0 / float(np.sqrt(D_K))

_CACHE = {}


def _build_program():
    """Build + compile the per-core Bass program (identical on all cores)."""
    if "nc" in _CACHE:
        return _CACHE["nc"]

    from concourse import bacc, mybir
    import concourse.tile as tile
    from concourse.masks import make_identity

    f32 = mybir.dt.float32
    f32r = mybir.dt.float32r
    AF = mybir.ActivationFunctionType
    ALU = mybir.AluOpType
    AX = mybir.AxisListType

    nc = bacc.Bacc("TRN2", target_bir_lowering=False, debug=False,
                   num_devices=N_CORES)

    xt_d = nc.dram_tensor("xt", [128, 32], f32r, kind="ExternalInput")
    wqt_d = nc.dram_tensor("wqt", [D_MODEL, MPC], f32r, kind="ExternalInput")
    wkt_d = nc.dram_tensor("wkt", [D_MODEL, MPC], f32r, kind="ExternalInput")
    wvt_d = nc.dram_tensor("wvt", [D_MODEL, MPC], f32r, kind="ExternalInput")
    wot_d = nc.dram_tensor("wot", [MPC, D_MODEL], f32r, kind="ExternalInput")
    kt_d = nc.dram_tensor("kt", [HPC, D_K, S], f32r, kind="ExternalInput")
    v_d = nc.dram_tensor("v", [128, (S // 128) * HPC * D_K], f32r, kind="ExternalInput")
    out_d = nc.dram_tensor("out", [1, D_MODEL], f32, kind="ExternalOutput")

    NT = S // 128                 # 32 seq tiles
    NC = S // 512                 # 8 512-wide chunks
    WDMA = 2048                   # free-size of one 1MiB weight DMA tile

    with tile.TileContext(nc) as tc:
        with (
            tc.tile_pool(name="singles", bufs=1) as singles,
            tc.tile_pool(name="wpool", bufs=5) as wpool,
            tc.tile_pool(name="kpool", bufs=4) as kpool,
            tc.tile_pool(name="vpool", bufs=1) as vpool,
            tc.tile_pool(name="outc", bufs=2) as outc,
            tc.tile_pool(name="prow_pool", bufs=8) as prow_pool,
            tc.tile_pool(name="rows", bufs=5, space="PSUM") as rows,
            tc.tile_pool(name="tp", bufs=2, space="PSUM") as tp,
            tc.tile_pool(name="av4p", bufs=1, space="PSUM") as av4p,
        ):
            # ---- constants / input vector ----
            xt = singles.tile([128, 32], f32r, tag="xt")
            nc.sync.dma_start(xt[:], xt_d.ap())
            ident = singles.tile([128, 128], f32, tag="ident")
            make_identity(nc, ident[:])
            ones_col = singles.tile([128, 1], f32, tag="ones_col")
            nc.vector.memset(ones_col[:], 1.0)

            qsb = singles.tile([128, HPC], f32r, tag="qsb")
            ksb = singles.tile([128, HPC], f32r, tag="ksb")
            qrow = singles.tile([1, MPC], f32, tag="qrow")
            krow = singles.tile([1, MPC], f32, tag="krow")
            vrow = singles.tile([1, MPC], f32, tag="vrow")
            p_all = singles.tile([128, HPC, NT], f32r, tag="p_all")
            rs4 = singles.tile([128, HPC], f32, tag="rs4")
            rec4 = singles.tile([HPC, 1], f32, tag="rec4")
            av4n = singles.tile([HPC, MPC], f32, tag="av4n")
            avn = singles.tile([128, HPC], f32r, tag="avn")

            def emit_proj(w_dram):
                """rows += x^T @ W_chunk; W is the moving operand (f32r)."""
                w_ap = w_dram.ap().rearrange("(b c p) m -> b p c m", c=4, p=128)
                acc = rows.tile([1, MPC], f32, tag="rows")
                for b in range(8):
                    wt = wpool.tile([128, WDMA], f32r, tag="wt")
                    wt_v = wt[:].rearrange("p (c m) -> p c m", c=4)
                    nc.sync.dma_start(wt_v, w_ap[b])
                    for c in range(4):
                        t = b * 4 + c
                        nc.tensor.matmul(
                            acc[:], xt[:, t:t + 1], wt_v[:, c, :],
                            start=(t == 0), stop=(t == NT - 1),
                            skip_group_check=True)
                return acc

            def transpose_row_to_cols(row_sb, dst_psum, cols):
                """(1,128) slices of row_sb -> columns `cols` of dst_psum."""
                for i, cc in enumerate(cols):
                    nc.tensor.matmul(dst_psum[:, cc:cc + 1],
                                     row_sb[0:1, i * 128:(i + 1) * 128],
                                     ident[0:1, 0:1], is_transpose=True,
                                     skip_group_check=True)

            def emit_score_chunks(h, kth, chunks):
                for c in chunks:
                    sc = rows.tile([1, 512], f32, tag="rows")
                    nc.tensor.matmul(sc[:], qsb[:, h:h + 1],
                                     kth[:, c * 512:(c + 1) * 512],
                                     skip_group_check=True)
                    prow = prow_pool.tile([1, 512], f32, tag="prow")
                    nc.scalar.activation(prow[:], sc[:], AF.Exp)
                    if c == NC - 1:
                        # stale cached entry at s = S-1: force prob 0 here;
                        # the true k_new/v_new contribution is added as a
                        # rank-1 term to A@V and to the sumexp
                        nc.vector.memset(prow[0:1, 511:512], 0.0)
                    yield prow
                # transposes emitted after all chunk MMs so the PE never
                # waits on the scalar engine mid-stream

            def emit_p_transposes(h, chunk_prows):
                for c, prow in chunk_prows:
                    ptp = tp.tile([128, 4], f32, tag="tp")
                    for i in range(4):
                        nc.tensor.matmul(ptp[:, i:i + 1],
                                         prow[0:1, i * 128:(i + 1) * 128],
                                         ident[0:1, 0:1], is_transpose=True,
                                         skip_group_check=True)
                    nc.vector.tensor_copy(p_all[:, h, c * 4:(c + 1) * 4],
                                          ptp[:])

            # ---- phase 1: W_Q stream + q projection, then q -> columns ----
            q_acc = emit_proj(wqt_d)

            # ---- phase 2: K^T streams right behind W_Q ----
            kths = []
            for h in range(HPC):
                kth = kpool.tile([128, S], f32r, tag="kth")
                nc.sync.dma_start(kth[:], kt_d.ap()[h])
                kths.append(kth)

            nc.vector.tensor_scalar_mul(qrow[:], q_acc[:], INV_SQRT_DK)
            q_t = tp.tile([128, HPC], f32, tag="tp")
            transpose_row_to_cols(qrow, q_t, range(HPC))
            nc.vector.tensor_copy(qsb[:], q_t[:])

            # ---- phase 3: scores/exp/transpose, all chunks ----
            for h in range(HPC):
                prows = list(emit_score_chunks(h, kths[h], range(NC)))
                emit_p_transposes(h, list(zip(range(NC), prows)))
                nc.vector.tensor_reduce(rs4[:, h:h + 1],
                                        p_all[:, h, :].bitcast(f32),
                                        axis=AX.X, op=ALU.add)

            # ---- phase 4: W_K / W_V streams + k/v projections ----
            k_acc = emit_proj(wkt_d)
            v_acc = emit_proj(wvt_d)
            nc.vector.tensor_copy(krow[:], k_acc[:])
            nc.vector.tensor_copy(vrow[:], v_acc[:])
            k_t = tp.tile([128, HPC], f32, tag="tp")
            transpose_row_to_cols(krow, k_t, range(HPC))
            nc.vector.tensor_copy(ksb[:], k_t[:])

            # ---- phase 5: V stream (4 x 2 MiB, fully contiguous) ----
            v4 = vpool.tile([128, NT * HPC * D_K], f32r, tag="v4")
            for piece in range(4):
                nc.sync.dma_start(
                    v4[:, piece * 4096:(piece + 1) * 4096],
                    v_d.ap()[:, piece * 4096:(piece + 1) * 4096])

            # ---- phase 6: p4095_h = exp(q_h . k_new_h / sqrt(dk)) ----
            sc4 = tp.tile([1, HPC], f32, tag="tp")
            for h in range(HPC):
                nc.tensor.matmul(sc4[:, h:h + 1],
                                 ksb[:, h:h + 1].bitcast(f32),
                                 qsb[:, h:h + 1].bitcast(f32),
                                 skip_group_check=True)
            p4095 = singles.tile([1, HPC], f32, tag="p4095")
            nc.scalar.activation(p4095[:], sc4[:], AF.Exp)

            # ---- phase 7: A@V head-batched ----
            av4 = av4p.tile([HPC, HPC * D_K], f32, tag="av4")
            for t in range(NT):
                nc.tensor.matmul(av4[:], p_all[:, :, t],
                                 v4[:, t * 512:(t + 1) * 512],
                                 start=(t == 0), stop=False,
                                 skip_group_check=True)
            # av4[g, (h,d)] += p4095_g * v_new_h[d]; diagonal g==h is the
            # true s = S-1 contribution (fp32, K=1 rank-1 update)
            nc.tensor.matmul(av4[:], p4095[:], vrow[:],
                             start=False, stop=True, skip_group_check=True)

            se = tp.tile([HPC, 1], f32, tag="tp")
            nc.tensor.matmul(se[:], rs4[:], ones_col[:],
                             start=True, stop=False, skip_group_check=True)
            nc.tensor.matmul(se[:], p4095[:], ones_col[0:1, :],
                             start=False, stop=True, skip_group_check=True)
            nc.vector.reciprocal(rec4[:], se[:])
            nc.vector.tensor_scalar_mul(av4n[:], av4[:], rec4[:, 0:1])

            # extract diagonal 128-blocks as columns
            for g in range(HPC):
                avt = tp.tile([128, HPC], f32, tag="tp")
                nc.tensor.matmul(avt[:], av4n[0:HPC, g * 128:(g + 1) * 128],
                                 ident[0:HPC, 0:HPC], is_transpose=True,
                                 skip_group_check=True)
                nc.vector.tensor_copy(avn[:, g:g + 1], avt[:, g:g + 1])

            # ---- phase 8: W_O stream + partial output ----
            wot_ap = wot_d.ap().rearrange("(hh p) (t j) -> hh t p j",
                                          p=128, j=WDMA)
            for b in range(2):
                wts = []
                for h in range(HPC):
                    wt = wpool.tile([128, WDMA], f32r, tag="wt")
                    nc.sync.dma_start(wt[:], wot_ap[h, b])
                    wts.append(wt)
                for jj in range(4):
                    jc = b * 4 + jj
                    wo_ps = rows.tile([1, 512], f32, tag="rows")
                    for h in range(HPC):
                        nc.tensor.matmul(
                            wo_ps[:], avn[:, h:h + 1],
                            wts[h][:, jj * 512:(jj + 1) * 512],
                            start=(h == 0), stop=(h == HPC - 1),
                            skip_group_check=True)
                    oc = outc.tile([1, 512], f32, tag="oc")
                    nc.vector.tensor_copy(oc[:], wo_ps[:])
                    nc.sync.dma_start(out_d.ap()[:, jc * 512:(jc + 1) * 512],
                                      oc[:])

    nc.compile()
    _CACHE["nc"] = nc
    return nc


def _numpy_reference(x, seq, pos, k_cache, v_cache, W_Q, W_K, W_V, W_O):
    """Fallback for shapes the compiled program doesn't cover."""
    xf = x.reshape(-1).astype(np.float32)
    q = (W_Q @ xf).reshape(N_HEADS, D_K)
    k_new = (W_K @ xf).reshape(N_HEADS, D_K)
    v_new = (W_V @ xf).reshape(N_HEADS, D_K)
    K = np.array(k_cache[seq, :pos + 1], dtype=np.float32)
    V = np.array(v_cache[seq, :pos + 1], dtype=np.float32)
    K[pos] = k_new
    V[pos] = v_new
    scores = np.einsum("hd,shd->hs", q, K) / np.float32(np.sqrt(D_K))
    scores -= scores.max(axis=-1, keepdims=True)
    e = np.exp(scores)
    attn = e / e.sum(axis=-1, keepdims=True)
    out = np.einsum("hs,shd->hd", attn, V).reshape(-1)
    return (W_O @ out).reshape(1, 1, D_MODEL).astype(np.float32)


def _make_in_maps(x, seq, k_cache, v_cache, W_Q, W_K, W_V, W_O):
    xt = np.ascontiguousarray(x.reshape(32, 128).T)
    k_seq = np.asarray(k_cache[seq], dtype=np.float32)   # (S, H, dk)
    v_seq = np.asarray(v_cache[seq], dtype=np.float32)
    in_maps = []
    for c in range(N_CORES):
        sl = slice(c * MPC, (c + 1) * MPC)
        hs = slice(c * HPC, (c + 1) * HPC)
        in_maps.append({
            "xt": xt,
            "wqt": np.ascontiguousarray(W_Q[sl, :].T),
            "wkt": np.ascontiguousarray(W_K[sl, :].T),
            "wvt": np.ascontiguousarray(W_V[sl, :].T),
            "wot": np.ascontiguousarray(W_O[:, sl].T),
            "kt": np.ascontiguousarray(k_seq[:, hs, :].transpose(1, 2, 0)),
            # (s_lo, (s_hi, head, d)) — matches the SBUF tile layout exactly,
            # so the 8 MiB V transfer is a single fully-contiguous DMA
            "v": np.ascontiguousarray(
                v_seq[:, hs, :].reshape(32, 128, HPC, D_K)
                .transpose(1, 0, 2, 3).reshape(128, 32 * HPC * D_K)),
        })
    return in_maps


def kernel(x, seq_idx, current_pos, k_cache, v_cache, W_Q, W_K, W_V, W_O):
    x = np.asarray(x, dtype=np.float32)
    k_cache = np.asarray(k_cache)
    v_cache = np.asarray(v_cache)
    W_Q = np.asarray(W_Q, dtype=np.float32)
    W_K = np.asarray(W_K, dtype=np.float32)
    W_V = np.asarray(W_V, dtype=np.float32)
    W_O = np.asarray(W_O, dtype=np.float32)
    seq = int(np.asarray(seq_idx))
    pos = int(np.asarray(current_pos))

    if pos != S - 1 or x.size != D_MODEL or k_cache.shape[1:] != (S, N_HEADS, D_K):
        return _numpy_reference(x, seq, pos, k_cache, v_cache, W_Q, W_K, W_V, W_O)

    from concourse.bass_utils import run_bass_kernel_spmd

    nc = _build_program()
    in_maps = _make_in_maps(x, seq, k_cache, v_cache, W_Q, W_K, W_V, W_O)

    last_err = None
    for _attempt in range(3):
        try:
            res = run_bass_kernel_spmd(nc, in_maps, core_ids=list(range(N_CORES)))
            break
        except Exception as e:          # transient NRT device errors
            last_err = e
    else:
        raise last_err

    y = np.zeros(D_MODEL, dtype=np.float32)
    for c in range(N_CORES):
        y += res.results[c]["out"].reshape(D_MODEL)
    return y.reshape(1, 1, D_MODEL)
